# revision 18
# baseline (speedup 1.0000x reference)
"""Bahdanau attention weights kernel for 8 Trainium2 NeuronCores.

Reference computation (per full input):
    proj_enc = encoder_output @ W1_w + W1_b            # [B,S,U]
    proj_h   = last_layer_h_n @ W2_w + W2_b            # [B,1,U]
    score    = tanh(proj_enc + proj_h) @ V_w + V_b     # [B,S,1]
    out      = softmax(score, axis=1)                  # [B,S,1]

Sharding: data-parallel over batch. Each of the 8 cores gets B/8 batches;
weights are replicated; softmax is over the local sequence axis, so no
cross-core communication is needed.

Per-core strategy (fp8 fast path):
  - The dominant cost is X @ W1 ([8192 tok, 1024] @ [1024, 1024]). Both
    operands are pre-quantized to fp8 e4m3 on the host (W1 scaled by 16
    so its U(-1/32,1/32) entries stay in the e4m3 normal range) and the
    matmul runs in DoubleRow perf mode: two h-blocks are contracted per
    instruction at 2x the bf16 PE rate. The 1/16 is folded into the tanh
    activation's scale operand.
  - fp8 quantization alone costs ~2.1e-2 end-to-end rel err. A host-side
    first-order correction recovers most of it: the score error is
    approximately mean(tanh') * (proj_err @ V), and proj_err @ V =
    xq @ (W1q @ V) - x @ (W1 @ V) needs only two O(tok*H) host matvecs
    (same complexity as the host transpose). The per-token correction
    m*c - V_b is DMA'd in and subtracted from the scores before the exp.
    This lands ~1.1e-2 rel err vs the 2e-2 gate.
  - The tiny bias row h_n @ W2 + b1 + b2 ([4, 1024]) is also computed on
    the host (0.05% of the FLOPs); the device gets it as a [u, b] f32
    table feeding the tanh's per-partition bias operand.
  - tanh runs on the scalar engine reading PSUM; the V contraction runs
    on the DVE as acc += V_ub (.) tanh_ub (per-partition scalar).
  - Scores are materialized TRANSPOSED: per 128-token chunk, a matmul
    with the f32r acc chunk as the stationary operand and an all-ones
    column as the moving operand gives score[t_p, 1] — tokens on
    partitions. A batch's 2048 scores form one [128, 16] tile, so the
    whole softmax tail (correction sub, exp, sum, reciprocal, scale) is
    a handful of full-width ops instead of [1, 2048] single-lane work.
    The normalized tile is PE-transposed back to [16, 128] and DMA'd out
    contiguously. Tail ops of batch b are staggered across the next
    group's ub slots so the in-order PE queue never waits on them.
"""

import sys

for _p in ("/opt/trn_rl_repo", "/root/.axon_site/_ro/trn_rl_repo"):
    if _p not in sys.path:
        sys.path.append(_p)

import numpy as np

import concourse.bacc as bacc
import concourse.tile as tile
from concourse import mybir
from concourse.masks import make_identity

F32 = mybir.dt.float32
F32R = mybir.dt.float32r
BF16 = mybir.dt.bfloat16
F8 = mybir.dt.float8e4

B, S, H, U = 32, 2048, 1024, 1024
N_CORES = 8
B_LOCAL = B // N_CORES  # 4
P = 128
T_GROUP = 512  # tokens per group (matmul moving dim)

W1_SCALE = 16.0  # host multiplies W1 by this before the e4m3 cast
M_CORR = 0.675   # first-order correction gain ~ E[tanh'] (fit offline)


def build_kernel(b_local=B_LOCAL, s=S, h=H, u=U):
    """Build the per-core Bass program. Shape params must keep:
    s % T_GROUP == 0, h % 256 == 0, u % 128 == 0.

    Host-side contract: encoder_output arrives TRANSPOSED [h, tokens] in
    fp8 e4m3; W1_w is [h, u] fp8 e4m3 pre-scaled by W1_SCALE; bias is
    (h_n @ W2 + b1 + b2).T [u, b] f32; corr is the per-token score
    correction (m*c - V_b) in transposed-score layout [128, b*16] f32."""
    nc = bacc.Bacc()

    n_tok = b_local * s
    n_groups = n_tok // T_GROUP
    gpb = s // T_GROUP     # groups per batch
    HB = h // P            # h blocks
    HP = HB // 2           # h block pairs (DoubleRow contracts 2 per matmul)
    UB = u // P            # u blocks
    TSUB = T_GROUP // P    # 128-token chunks per group
    QCOLS = gpb * TSUB     # score columns per batch (16)

    n_g = n_tok // T_GROUP
    # all big inputs arrive pre-permuted so every DMA reads long
    # per-partition-contiguous lines (128 descriptors instead of 1024)
    enc = nc.dram_tensor(
        "encoder_output", [n_g * P, (h // P) * T_GROUP], F8, kind="ExternalInput")
    w1 = nc.dram_tensor("W1_w", [P, (h // P) * u], F8, kind="ExternalInput")
    vw = nc.dram_tensor("V_w", [P, u // P], F32, kind="ExternalInput")
    bias = nc.dram_tensor(
        "bias", [P, (u // P) * b_local], F32, kind="ExternalInput")
    corr = nc.dram_tensor("corr", [P, b_local * QCOLS], F32, kind="ExternalInput")
    out = nc.dram_tensor("out", [b_local, s], F32, kind="ExternalOutput")

    encT_v = enc.ap().rearrange("(g p) (hb t) -> g p hb t", p=P, t=T_GROUP)
    w1_v = w1.ap().rearrange("p (hb u) -> p hb u", u=u)
    out_v = out.ap().rearrange("b (q p) -> b q p", p=P)

    NPREF = 5
    XTBUFS = NPREF + 3

    with tile.TileContext(nc) as tc:
        with (
            tc.tile_pool(name="consts", bufs=1) as consts,
            tc.tile_pool(name="wpool", bufs=1) as wpool,
            tc.tile_pool(name="xtpool", bufs=XTBUFS) as xtpool,
            tc.tile_pool(name="thpool", bufs=4) as thpool,
            tc.tile_pool(name="scpool", bufs=3) as scpool,
            tc.tile_pool(name="smpool", bufs=2) as smpool,
            tc.tile_pool(name="psu", bufs=3, space="PSUM") as psu,
            tc.tile_pool(name="pssc", bufs=2, space="PSUM") as pssc,
            tc.tile_pool(name="pstail", bufs=1, space="PSUM") as pstail,
        ):
            # ---- constants -------------------------------------------------
            ident = consts.tile([P, P], F32)
            make_identity(nc, ident)
            identL = consts.tile([P, P], BF16)
            nc.vector.tensor_copy(identL, ident)

            # PE clock warm-up: dummy matmuls on the identity run inside the
            # initial DMA window, so the HAM un-throttles the PE before the
            # first real matmul (cold rate is half speed)
            for w in range(48):
                warm_ps = psu.tile([P, T_GROUP], F32, tag="pu")
                nc.tensor.matmul(warm_ps[:, :P], lhsT=identL, rhs=identL)

            # prefetch machinery for X^T tiles ([p, hb, t] fp8 from DRAM)
            PREFETCH = NPREF
            x_pending = {}

            def issue_x(g):
                xT = xtpool.tile([P, HB, T_GROUP], F8, tag="xT")
                nc.sync.dma_start(out=xT, in_=encT_v[g])
                x_pending[g] = xT

            # X(0) and W1 gate the main matmuls: issue them before every
            # other DMA (each dma_start costs ~0.7us of descriptor setup on
            # the sync queue, and a single start already fans out across
            # HW-DGE queues), and hold the deeper X prefetch back in the
            # main loop so it does not steal startup DMA bandwidth.
            issue_x(0)
            w1_sb = wpool.tile([P, HB, u], F8, tag="w1")
            nc.sync.dma_start(out=w1_sb, in_=w1_v)

            # tanh bias (host-precomputed h_n @ W2 + b1 + b2) [u_p, ub, b]
            bias_sb = consts.tile([P, UB, b_local], F32)
            nc.sync.dma_start(
                out=bias_sb, in_=bias.ap().rearrange("p (ub b) -> p ub b", b=b_local)
            )
            # V in [u_p, u_blk] layout, f32 (only the DVE reads it as a
            # per-partition scalar, which must be f32)
            v_sb = consts.tile([P, UB], F32)
            nc.sync.dma_start(out=v_sb, in_=vw.ap())
            next_x = 1

            # all-ones column/row for partition sums and broadcasts
            ones_col = consts.tile([P, 1], F32)
            nc.vector.memset(ones_col, 1.0)
            ones16 = consts.tile([P, 1], BF16)
            nc.vector.memset(ones16, 1.0)
            v16 = consts.tile([P, UB], BF16)
            nc.vector.tensor_copy(v16, v_sb)
            ones_row = consts.tile([1, P], F32)
            nc.vector.memset(ones_row, 1.0)

            # per-token correction (m*c - V_b) in [p, b*16] score layout
            corr_sb = consts.tile([P, b_local * QCOLS], F32)
            nc.sync.dma_start(out=corr_sb, in_=corr.ap())

            # ---- main loop over token groups ------------------------------
            # Deferred finish: group g's score matmuls are emitted after
            # group g+1's first matmul chain (so the PE never waits on the
            # DVE accumulation), and the batch softmax tail is staggered
            # across later ub slots.
            state = {"pending": None, "score": None, "tail": []}

            def emit_tsums(acc16, b, gi):
                # score[t, 1] per 128-token chunk: stationary = bf16 acc
                # chunk, moving = all-ones column (sums the 128 partitions)
                if gi == 0:
                    state["score"] = pssc.tile(
                        [P, QCOLS], F32, tag="score", name=f"score_{b}")
                score_ps = state["score"]
                for i in range(TSUB):
                    nc.tensor.matmul(
                        score_ps[:, gi * TSUB + i : gi * TSUB + i + 1],
                        lhsT=acc16[:, i * P : (i + 1) * P],
                        rhs=ones16,
                    )
                if gi == gpb - 1:
                    queue_tail(b, score_ps)

            def queue_tail(b, score_ps):
                # softmax over the batch's [128, 16] transposed score tile;
                # stages are emitted one ub-slot apart so every PE op's
                # inputs are ready when the in-order PE queue reaches it
                bc = slice(b * QCOLS, (b + 1) * QCOLS)
                sub_sb = smpool.tile([P, QCOLS], F32, tag="sub")
                exp_sb = smpool.tile([P, QCOLS], F32, tag="exp")
                esum = smpool.tile([P, 1], F32, tag="esum")
                tot_ps = pstail.tile([1, 1], F32, tag="tot")
                rec_sb = smpool.tile([1, 1], F32, tag="rec")
                rec_ps = pstail.tile([QCOLS, 1], F32, tag="recb")
                rec16 = smpool.tile([QCOLS, 1], F32, tag="rec16")
                tr_ps = pstail.tile([QCOLS, P], F32, tag="tr")
                outT = smpool.tile([QCOLS, P], F32, tag="outT")

                def s1():
                    nc.vector.tensor_sub(sub_sb, score_ps, corr_sb[:, bc])
                    nc.scalar.activation(
                        exp_sb, sub_sb,
                        mybir.ActivationFunctionType.Exp,
                        accum_out=esum,
                    )

                def s2():
                    # transpose of the unnormalized exps overlaps the
                    # denominator chain; the normalization is folded into
                    # the PSUM->SBUF copy via the activation scale operand
                    nc.tensor.matmul(tot_ps, lhsT=ones_col, rhs=esum)
                    nc.tensor.transpose(tr_ps, exp_sb, ident)
                    nc.vector.reciprocal(rec_sb, tot_ps)

                def s3():
                    nc.tensor.matmul(
                        rec_ps, lhsT=ones_row[:, :QCOLS], rhs=rec_sb,
                    )
                    nc.vector.tensor_copy(rec16, rec_ps)

                def s4():
                    nc.scalar.activation(
                        outT, tr_ps,
                        mybir.ActivationFunctionType.Copy,
                        scale=rec16,
                    )
                    nc.sync.dma_start(out=out_v[b], in_=outT)

                state["tail"] = [s1, s2, s3, s4]

            for g in range(n_groups):
                b = g // gpb
                gi = g % gpb

                issued = 0
                while next_x < min(g + PREFETCH + 1, n_groups) and issued < 2:
                    issue_x(next_x)
                    next_x += 1
                    issued += 1

                xT = x_pending.pop(g)

                # proj^T[u, t] blocks + tanh; the V contraction runs on
                # the DVE as acc += V_ub (.) tanh_ub (per-partition scalar)
                acc = scpool.tile([P, T_GROUP], F32, tag="acc")
                acc16 = scpool.tile([P, T_GROUP], BF16, tag="acc16")
                for ub in range(UB):
                    pu = psu.tile([P, T_GROUP], F32, tag="pu")
                    for hp in range(HP):
                        nc.tensor.matmul(
                            pu,
                            lhsT=w1_sb[:, 2 * hp : 2 * hp + 2, ub * P : (ub + 1) * P],
                            rhs=xT[:, 2 * hp : 2 * hp + 2, :],
                            start=(hp == 0),
                            stop=(hp == HP - 1),
                            perf_mode=mybir.MatmulPerfMode.DoubleRow,
                        )
                    th = thpool.tile([P, T_GROUP], BF16, tag="th")
                    nc.scalar.activation(
                        th, pu,
                        mybir.ActivationFunctionType.Tanh,
                        bias=bias_sb[:, ub, b : b + 1],
                        scale=1.0 / W1_SCALE,
                    )
                    last_g = g == n_groups - 1
                    acc16_ub = UB - 2 if last_g else UB - 1
                    if ub == 0:
                        nc.vector.tensor_scalar_mul(acc, th, v_sb[:, 0:1])
                    elif not (last_g and ub == UB - 1):
                        # the final accumulation writes bf16: one rounding,
                        # same precision as a separate bf16 copy but free
                        nc.vector.scalar_tensor_tensor(
                            acc16 if ub == acc16_ub else acc,
                            th, v_sb[:, ub : ub + 1], acc,
                            op0=mybir.AluOpType.mult,
                            op1=mybir.AluOpType.add,
                        )
                        if ub == 2 and state["pending"] is not None:
                            emit_tsums(*state["pending"])
                            state["pending"] = None
                        elif ub >= 3 and state["tail"]:
                            state["tail"].pop(0)()
                    else:
                        # closing chain: the last u-block's V-term goes
                        # straight into the score sums (second accumulating
                        # matmul, th stationary / bf16 V column moving)
                        # instead of through another full-width DVE pass
                        score_ps = state["score"]
                        for i in range(TSUB):
                            cs = slice(i * P, (i + 1) * P)
                            col = gi * TSUB + i
                            nc.tensor.matmul(
                                score_ps[:, col : col + 1],
                                lhsT=acc16[:, cs], rhs=ones16,
                                start=True, stop=False,
                            )
                            nc.tensor.matmul(
                                score_ps[:, col : col + 1],
                                lhsT=th[:, cs],
                                rhs=v16[:, UB - 1 : UB],
                                start=False, stop=True,
                            )
                        queue_tail(b, score_ps)
                        for st in state["tail"]:
                            st()
                        state["tail"] = []
                state["pending"] = (acc16, b, gi) if g < n_groups - 1 else None

            assert state["pending"] is None and not state["tail"]

    nc.compile()
    return nc


def make_in_maps(inputs):
    """Shard the full inputs per core. encoder_output / W1_w are cast to
    fp8 e4m3 on the host (W1 pre-scaled by W1_SCALE); encoder_output is
    pre-transposed to [H, tokens]. The bias row h_n @ W2 + b1 + b2 and
    the first-order score correction m*c - V_b are host-precomputed
    (two O(tok*H) matvecs, same complexity as the transpose)."""
    import ml_dtypes

    e4m3 = ml_dtypes.float8_e4m3fn

    def f32(name):
        return np.ascontiguousarray(np.asarray(inputs[name], dtype=np.float32))

    enc = f32("encoder_output")          # [B, S, H]
    hn = f32("last_layer_h_n")
    w1 = f32("W1_w")
    w2 = f32("W2_w")
    vw = f32("V_w")
    b1, b2, vb = f32("W1_b"), f32("W2_b"), f32("V_b")

    w1q = (w1 * np.float32(W1_SCALE)).astype(e4m3)
    encq = enc.reshape(B * S, H).astype(e4m3)

    # tanh bias table [B, U]
    bias_full = hn @ w2 + b1 + b2

    # first-order fp8 correction: c_t = (proj_q - proj)[t] @ V
    w1v_q = (w1q.astype(np.float32) @ vw[:, 0]) / np.float32(W1_SCALE)
    w1v = w1.astype(np.float64) @ vw[:, 0].astype(np.float64)
    c = (encq.astype(np.float32) @ w1v_q
         - (enc.reshape(B * S, H) @ w1v.astype(np.float32)))
    mc = (np.float32(M_CORR) * c - vb[0]).reshape(B, S)
    # transposed-score layout: [b][gi][i][p] -> [p, b*16 + gi*4 + i]
    gpb = S // T_GROUP
    tsub = T_GROUP // P
    mcT = mc.reshape(B, gpb, tsub, P).transpose(3, 0, 1, 2).reshape(P, B * gpb * tsub)

    # per-partition-contiguous device layouts
    HB = H // P
    w1_dev = np.ascontiguousarray(
        w1q.reshape(HB, P, U).transpose(1, 0, 2).reshape(P, HB * U))
    v_dev = np.ascontiguousarray(vw[:, 0].reshape(U // P, P).T)
    in_maps = []
    for cid in range(N_CORES):
        sl = slice(cid * B_LOCAL, (cid + 1) * B_LOCAL)
        n_g = B_LOCAL * S // T_GROUP
        e = (encq.reshape(B, S, H)[sl]
             .reshape(n_g, T_GROUP, HB, P)       # [g][t][hb][p]
             .transpose(0, 3, 2, 1)              # [g][p][hb][t]
             .reshape(n_g * P, HB * T_GROUP))
        bias_dev = (bias_full[sl].T              # [u, b]
                    .reshape(U // P, P, B_LOCAL)
                    .transpose(1, 0, 2)
                    .reshape(P, (U // P) * B_LOCAL))
        in_maps.append({
            "encoder_output": np.ascontiguousarray(e),
            "W1_w": w1_dev,
            "V_w": v_dev,
            "bias": np.ascontiguousarray(bias_dev),
            "corr": np.ascontiguousarray(
                mcT[:, cid * B_LOCAL * gpb * tsub : (cid + 1) * B_LOCAL * gpb * tsub]),
        })
    return in_maps


def kernel(**inputs):
    from concourse.bass_utils import run_bass_kernel_spmd

    nc = build_kernel()
    in_maps = make_in_maps(inputs)
    res = run_bass_kernel_spmd(nc, in_maps, core_ids=list(range(N_CORES)))
    outs = [res.results[c]["out"].reshape(B_LOCAL, S, 1) for c in range(N_CORES)]
    return np.concatenate(outs, axis=0)


# revision 20
# speedup vs baseline: 1.0013x; 1.0013x over previous
"""Bahdanau attention weights kernel for 8 Trainium2 NeuronCores.

Reference computation (per full input):
    proj_enc = encoder_output @ W1_w + W1_b            # [B,S,U]
    proj_h   = last_layer_h_n @ W2_w + W2_b            # [B,1,U]
    score    = tanh(proj_enc + proj_h) @ V_w + V_b     # [B,S,1]
    out      = softmax(score, axis=1)                  # [B,S,1]

Sharding: data-parallel over batch. Each of the 8 cores gets B/8 batches;
weights are replicated; softmax is over the local sequence axis, so no
cross-core communication is needed.

Per-core strategy (fp8 fast path):
  - The dominant cost is X @ W1 ([8192 tok, 1024] @ [1024, 1024]). Both
    operands are pre-quantized to fp8 e4m3 on the host (W1 scaled by 16
    so its U(-1/32,1/32) entries stay in the e4m3 normal range) and the
    matmul runs in DoubleRow perf mode: two h-blocks are contracted per
    instruction at 2x the bf16 PE rate. The 1/16 is folded into the tanh
    activation's scale operand.
  - fp8 quantization alone costs ~2.1e-2 end-to-end rel err. A host-side
    first-order correction recovers most of it: the score error is
    approximately mean(tanh') * (proj_err @ V), and proj_err @ V =
    xq @ (W1q @ V) - x @ (W1 @ V) needs only two O(tok*H) host matvecs
    (same complexity as the host transpose). The per-token correction
    m*c - V_b is DMA'd in and subtracted from the scores before the exp.
    This lands ~1.1e-2 rel err vs the 2e-2 gate.
  - The tiny bias row h_n @ W2 + b1 + b2 ([4, 1024]) is also computed on
    the host (0.05% of the FLOPs); the device gets it as a [u, b] f32
    table feeding the tanh's per-partition bias operand.
  - tanh runs on the scalar engine reading PSUM; the V contraction runs
    on the DVE as acc += V_ub (.) tanh_ub (per-partition scalar).
  - Scores are materialized TRANSPOSED: per 128-token chunk, a matmul
    with the f32r acc chunk as the stationary operand and an all-ones
    column as the moving operand gives score[t_p, 1] — tokens on
    partitions. A batch's 2048 scores form one [128, 16] tile, so the
    whole softmax tail (correction sub, exp, sum, reciprocal, scale) is
    a handful of full-width ops instead of [1, 2048] single-lane work.
    The normalized tile is PE-transposed back to [16, 128] and DMA'd out
    contiguously. Tail ops of batch b are staggered across the next
    group's ub slots so the in-order PE queue never waits on them.
"""

import sys

for _p in ("/opt/trn_rl_repo", "/root/.axon_site/_ro/trn_rl_repo"):
    if _p not in sys.path:
        sys.path.append(_p)

import numpy as np

import concourse.bacc as bacc
import concourse.tile as tile
from concourse import mybir
from concourse.masks import make_identity

F32 = mybir.dt.float32
F32R = mybir.dt.float32r
BF16 = mybir.dt.bfloat16
F8 = mybir.dt.float8e4

B, S, H, U = 32, 2048, 1024, 1024
N_CORES = 8
B_LOCAL = B // N_CORES  # 4
P = 128
T_GROUP = 512  # tokens per group (matmul moving dim)

W1_SCALE = 16.0  # host multiplies W1 by this before the e4m3 cast
M_CORR = 0.675   # first-order correction gain ~ E[tanh'] (fit offline)


def build_kernel(b_local=B_LOCAL, s=S, h=H, u=U):
    """Build the per-core Bass program. Shape params must keep:
    s % T_GROUP == 0, h % 256 == 0, u % 128 == 0.

    Host-side contract: encoder_output arrives TRANSPOSED [h, tokens] in
    fp8 e4m3; W1_w is [h, u] fp8 e4m3 pre-scaled by W1_SCALE; bias is
    (h_n @ W2 + b1 + b2).T [u, b] f32; corr is the per-token score
    correction (m*c - V_b) in transposed-score layout [128, b*16] f32."""
    nc = bacc.Bacc()

    n_tok = b_local * s
    n_groups = n_tok // T_GROUP
    gpb = s // T_GROUP     # groups per batch
    HB = h // P            # h blocks
    HP = HB // 2           # h block pairs (DoubleRow contracts 2 per matmul)
    UB = u // P            # u blocks
    TSUB = T_GROUP // P    # 128-token chunks per group
    QCOLS = gpb * TSUB     # score columns per batch (16)

    n_g = n_tok // T_GROUP
    # all big inputs arrive pre-permuted so every DMA reads long
    # per-partition-contiguous lines (128 descriptors instead of 1024)
    enc = nc.dram_tensor(
        "encoder_output", [n_g * P, (h // P) * T_GROUP], F8, kind="ExternalInput")
    w1 = nc.dram_tensor("W1_w", [P, (h // P) * u], F8, kind="ExternalInput")
    vw = nc.dram_tensor("V_w", [P, u // P], F32, kind="ExternalInput")
    bias = nc.dram_tensor(
        "bias", [P, (u // P) * b_local], F32, kind="ExternalInput")
    corr = nc.dram_tensor("corr", [P, b_local * QCOLS], F32, kind="ExternalInput")
    out = nc.dram_tensor("out", [b_local, s], F32, kind="ExternalOutput")

    encT_v = enc.ap().rearrange("(g p) (hb t) -> g p hb t", p=P, t=T_GROUP)
    w1_v = w1.ap().rearrange("p (hb u) -> p hb u", u=u)
    out_v = out.ap().rearrange("b (q p) -> b q p", p=P)

    NPREF = 5
    XTBUFS = NPREF + 3

    with tile.TileContext(nc) as tc:
        with (
            tc.tile_pool(name="consts", bufs=1) as consts,
            tc.tile_pool(name="wpool", bufs=1) as wpool,
            tc.tile_pool(name="xtpool", bufs=XTBUFS) as xtpool,
            tc.tile_pool(name="thpool", bufs=4) as thpool,
            tc.tile_pool(name="scpool", bufs=3) as scpool,
            tc.tile_pool(name="smpool", bufs=2) as smpool,
            tc.tile_pool(name="psu", bufs=3, space="PSUM") as psu,
            tc.tile_pool(name="pssc", bufs=2, space="PSUM") as pssc,
            tc.tile_pool(name="pstail", bufs=1, space="PSUM") as pstail,
        ):
            # ---- constants -------------------------------------------------
            ident = consts.tile([P, P], F32)
            make_identity(nc, ident)
            identL = consts.tile([P, P], BF16)
            nc.vector.tensor_copy(identL, ident)

            # PE clock warm-up: dummy matmuls on the identity run inside the
            # initial DMA window, so the HAM un-throttles the PE before the
            # first real matmul (cold rate is half speed)
            for w in range(48):
                warm_ps = psu.tile([P, T_GROUP], F32, tag="pu")
                nc.tensor.matmul(warm_ps[:, :P], lhsT=identL, rhs=identL)

            # prefetch machinery for X^T tiles ([p, hb, t] fp8 from DRAM)
            PREFETCH = NPREF
            x_pending = {}

            def issue_x(g):
                xT = xtpool.tile([P, HB, T_GROUP], F8, tag="xT")
                nc.sync.dma_start(out=xT, in_=encT_v[g])
                x_pending[g] = xT

            # X(0) and W1 gate the main matmuls: issue them before every
            # other DMA (each dma_start costs ~0.7us of descriptor setup on
            # the sync queue, and a single start already fans out across
            # HW-DGE queues), and hold the deeper X prefetch back in the
            # main loop so it does not steal startup DMA bandwidth.
            issue_x(0)
            w1_sb = wpool.tile([P, HB, u], F8, tag="w1")
            nc.sync.dma_start(out=w1_sb, in_=w1_v)

            # tanh bias (host-precomputed h_n @ W2 + b1 + b2) [u_p, ub, b]
            bias_sb = consts.tile([P, UB, b_local], F32)
            nc.sync.dma_start(
                out=bias_sb, in_=bias.ap().rearrange("p (ub b) -> p ub b", b=b_local)
            )
            # V in [u_p, u_blk] layout, f32 (only the DVE reads it as a
            # per-partition scalar, which must be f32)
            v_sb = consts.tile([P, UB], F32)
            nc.sync.dma_start(out=v_sb, in_=vw.ap())
            next_x = 1

            # all-ones column/row for partition sums and broadcasts
            ones_col = consts.tile([P, 1], F32)
            nc.vector.memset(ones_col, 1.0)
            ones16 = consts.tile([P, 1], BF16)
            nc.vector.memset(ones16, 1.0)
            v16 = consts.tile([P, UB], BF16)
            nc.vector.tensor_copy(v16, v_sb)
            ones_row = consts.tile([1, P], F32)
            nc.vector.memset(ones_row, 1.0)

            # per-token correction (m*c - V_b) in [p, b*16] score layout
            corr_sb = consts.tile([P, b_local * QCOLS], F32)
            nc.sync.dma_start(out=corr_sb, in_=corr.ap())

            # ---- main loop over token groups ------------------------------
            # Deferred finish: group g's score matmuls are emitted after
            # group g+1's first matmul chain (so the PE never waits on the
            # DVE accumulation), and the batch softmax tail is staggered
            # across later ub slots.
            state = {"pending": None, "score": None, "tail": []}

            def emit_tsums(acc16, b, gi):
                # score[t, 1] per 128-token chunk: stationary = bf16 acc
                # chunk, moving = all-ones column (sums the 128 partitions)
                if gi == 0:
                    state["score"] = pssc.tile(
                        [P, QCOLS], F32, tag="score", name=f"score_{b}")
                score_ps = state["score"]
                for i in range(TSUB):
                    nc.tensor.matmul(
                        score_ps[:, gi * TSUB + i : gi * TSUB + i + 1],
                        lhsT=acc16[:, i * P : (i + 1) * P],
                        rhs=ones16,
                    )
                if gi == gpb - 1:
                    queue_tail(b, score_ps)

            def queue_tail(b, score_ps):
                # softmax over the batch's [128, 16] transposed score tile;
                # stages are emitted one ub-slot apart so every PE op's
                # inputs are ready when the in-order PE queue reaches it
                bc = slice(b * QCOLS, (b + 1) * QCOLS)
                sub_sb = smpool.tile([P, QCOLS], F32, tag="sub")
                exp_sb = smpool.tile([P, QCOLS], F32, tag="exp")
                esum = smpool.tile([P, 1], F32, tag="esum")
                tot_ps = pstail.tile([1, 1], F32, tag="tot")
                rec_sb = smpool.tile([1, 1], F32, tag="rec")
                rec_ps = pstail.tile([QCOLS, 1], F32, tag="recb")
                rec16 = smpool.tile([QCOLS, 1], F32, tag="rec16")
                tr_ps = pstail.tile([QCOLS, P], F32, tag="tr")
                outT = smpool.tile([QCOLS, P], F32, tag="outT")

                def s1():
                    nc.vector.tensor_sub(sub_sb, score_ps, corr_sb[:, bc])
                    nc.scalar.activation(
                        exp_sb, sub_sb,
                        mybir.ActivationFunctionType.Exp,
                        accum_out=esum,
                    )

                def s2():
                    # transpose of the unnormalized exps overlaps the
                    # denominator chain; the normalization is folded into
                    # the PSUM->SBUF copy via the activation scale operand
                    nc.tensor.matmul(tot_ps, lhsT=ones_col, rhs=esum)
                    nc.tensor.transpose(tr_ps, exp_sb, ident)
                    nc.vector.reciprocal(rec_sb, tot_ps)

                def s3():
                    nc.tensor.matmul(
                        rec_ps, lhsT=ones_row[:, :QCOLS], rhs=rec_sb,
                    )
                    nc.vector.tensor_copy(rec16, rec_ps)

                def s4():
                    nc.scalar.activation(
                        outT, tr_ps,
                        mybir.ActivationFunctionType.Copy,
                        scale=rec16,
                    )
                    nc.sync.dma_start(out=out_v[b], in_=outT)

                state["tail"] = [s1, s2, s3, s4]

            for g in range(n_groups):
                b = g // gpb
                gi = g % gpb

                issued = 0
                while next_x < min(g + PREFETCH + 1, n_groups) and issued < 2:
                    issue_x(next_x)
                    next_x += 1
                    issued += 1

                xT = x_pending.pop(g)

                # proj^T[u, t] blocks + tanh; the V contraction runs on
                # the DVE as acc += V_ub (.) tanh_ub (per-partition scalar)
                acc = scpool.tile([P, T_GROUP], F32, tag="acc")
                acc16 = scpool.tile([P, T_GROUP], BF16, tag="acc16")
                for ub in range(UB):
                    pu = psu.tile([P, T_GROUP], F32, tag="pu")
                    for hp in range(HP):
                        nc.tensor.matmul(
                            pu,
                            lhsT=w1_sb[:, 2 * hp : 2 * hp + 2, ub * P : (ub + 1) * P],
                            rhs=xT[:, 2 * hp : 2 * hp + 2, :],
                            start=(hp == 0),
                            stop=(hp == HP - 1),
                            perf_mode=mybir.MatmulPerfMode.DoubleRow,
                        )
                    th = thpool.tile([P, T_GROUP], BF16, tag="th")
                    nc.scalar.activation(
                        th, pu,
                        mybir.ActivationFunctionType.Tanh,
                        bias=bias_sb[:, ub, b : b + 1],
                        scale=1.0 / W1_SCALE,
                    )
                    last_g = g == n_groups - 1
                    acc16_ub = UB - 2 if last_g else UB - 1
                    if ub == 0:
                        nc.vector.tensor_scalar_mul(acc, th, v_sb[:, 0:1])
                    elif not (last_g and ub == UB - 1):
                        # the final accumulation writes bf16: one rounding,
                        # same precision as a separate bf16 copy but free
                        nc.vector.scalar_tensor_tensor(
                            acc16 if ub == acc16_ub else acc,
                            th, v_sb[:, ub : ub + 1], acc,
                            op0=mybir.AluOpType.mult,
                            op1=mybir.AluOpType.add,
                        )
                        if ub == 2 and state["pending"] is not None:
                            emit_tsums(*state["pending"])
                            state["pending"] = None
                        elif ub >= 3 and state["tail"]:
                            state["tail"].pop(0)()
                    else:
                        # closing chain: the last u-block's V-term goes
                        # straight into the score sums (second accumulating
                        # matmul, th stationary / bf16 V column moving)
                        # instead of through another full-width DVE pass
                        score_ps = state["score"]
                        for i in range(TSUB):
                            cs = slice(i * P, (i + 1) * P)
                            col = gi * TSUB + i
                            nc.tensor.matmul(
                                score_ps[:, col : col + 1],
                                lhsT=acc16[:, cs], rhs=ones16,
                                start=True, stop=False,
                            )
                            nc.tensor.matmul(
                                score_ps[:, col : col + 1],
                                lhsT=th[:, cs],
                                rhs=v16[:, UB - 1 : UB],
                                start=False, stop=True,
                            )
                        queue_tail(b, score_ps)
                        for st in state["tail"]:
                            st()
                        state["tail"] = []
                state["pending"] = (acc16, b, gi) if g < n_groups - 1 else None

            assert state["pending"] is None and not state["tail"]

    nc.compile()
    return nc


def make_in_maps(inputs):
    """Shard the full inputs per core. encoder_output / W1_w are cast to
    fp8 e4m3 on the host (W1 pre-scaled by W1_SCALE); encoder_output is
    pre-transposed to [H, tokens]. The bias row h_n @ W2 + b1 + b2 and
    the first-order score correction m*c - V_b are host-precomputed
    (two O(tok*H) matvecs, same complexity as the transpose)."""
    import ml_dtypes

    e4m3 = ml_dtypes.float8_e4m3fn

    def f32(name):
        return np.ascontiguousarray(np.asarray(inputs[name], dtype=np.float32))

    enc = f32("encoder_output")          # [B, S, H]
    hn = f32("last_layer_h_n")
    w1 = f32("W1_w")
    w2 = f32("W2_w")
    vw = f32("V_w")
    b1, b2, vb = f32("W1_b"), f32("W2_b"), f32("V_b")

    w1q = (w1 * np.float32(W1_SCALE)).astype(e4m3)
    encq = enc.reshape(B * S, H).astype(e4m3)

    # tanh bias table [B, U]
    bias_full = hn @ w2 + b1 + b2

    # first-order fp8 correction: c_t = (proj_q - proj)[t] @ V
    w1v_q = (w1q.astype(np.float32) @ vw[:, 0]) / np.float32(W1_SCALE)
    w1v = w1.astype(np.float64) @ vw[:, 0].astype(np.float64)
    c = (encq.astype(np.float32) @ w1v_q
         - (enc.reshape(B * S, H) @ w1v.astype(np.float32)))
    mc = (np.float32(M_CORR) * c - vb[0]).reshape(B, S)
    # transposed-score layout: [b][gi][i][p] -> [p, b*16 + gi*4 + i]
    gpb = S // T_GROUP
    tsub = T_GROUP // P
    mcT = mc.reshape(B, gpb, tsub, P).transpose(3, 0, 1, 2).reshape(P, B * gpb * tsub)

    # per-partition-contiguous device layouts
    HB = H // P
    w1_dev = np.ascontiguousarray(
        w1q.reshape(HB, P, U).transpose(1, 0, 2).reshape(P, HB * U))
    v_dev = np.ascontiguousarray(vw[:, 0].reshape(U // P, P).T)
    in_maps = []
    for cid in range(N_CORES):
        sl = slice(cid * B_LOCAL, (cid + 1) * B_LOCAL)
        n_g = B_LOCAL * S // T_GROUP
        e = (encq.reshape(B, S, H)[sl]
             .reshape(n_g, T_GROUP, HB, P)       # [g][t][hb][p]
             .transpose(0, 3, 2, 1)              # [g][p][hb][t]
             .reshape(n_g * P, HB * T_GROUP))
        bias_dev = (bias_full[sl].T              # [u, b]
                    .reshape(U // P, P, B_LOCAL)
                    .transpose(1, 0, 2)
                    .reshape(P, (U // P) * B_LOCAL))
        in_maps.append({
            "encoder_output": np.ascontiguousarray(e),
            "W1_w": w1_dev,
            "V_w": v_dev,
            "bias": np.ascontiguousarray(bias_dev),
            "corr": np.ascontiguousarray(
                mcT[:, cid * B_LOCAL * gpb * tsub : (cid + 1) * B_LOCAL * gpb * tsub]),
        })
    return in_maps


def kernel(**inputs):
    from concourse.bass_utils import run_bass_kernel_spmd

    nc = build_kernel()
    in_maps = make_in_maps(inputs)
    res = run_bass_kernel_spmd(nc, in_maps, core_ids=list(range(N_CORES)))
    outs = [res.results[c]["out"].reshape(B_LOCAL, S, 1) for c in range(N_CORES)]
    return np.concatenate(outs, axis=0)
